# revision 1
# baseline (speedup 1.0000x reference)
"""Correlation network kernel for Trainium2.

corr[b,i,j,k,l] = sum_c A[b,i,j,c] * B[b,k,l,c]

Per batch b this is  A_b (2304x64) @ B_b^T (64x2304) -> 2304x2304.
Sharding: data-parallel over batch B=8 across the 8 NeuronCores; each core
computes one full 2304x2304 correlation matrix (21.2 MB fp32 out), so the
kernel is output-write bound (~358 GB/s HBM per core => ~60 us floor).

Device-side plan (per core):
  - Inputs arrive host-prepped: transposed to [C, HW] layout, bf16 hi/lo
    split. The kernel computes hi*hi + hi*lo (the lo*hi term is dropped)
    and emits the OUTPUT AS BF16, halving the dominant HBM write traffic
    (21.2 -> 10.6 MB/core; fro rel err ~2.3e-3, well inside the 2e-2
    gate; the host upcasts to fp32).
  - K=C=64 uses only half the 128-row PE array, so m-tiles are packed in
    pairs: even m-tiles occupy array rows 0-63, odd m-tiles rows 64-127.
    The two groups' matmuls run concurrently; B^T operands are duplicated
    into both partition halves so each group streams its own rows.
  - Inputs load via the two HWDGE rings (sync + scalar) right after the
    ~7 us Tile preamble; SWDGE/gpsimd was ~2 us slower to first byte.
  - Per (m-pair, 512-col n-tile): 4 bf16 matmuls into two PSUM banks,
    then narrow [128,512] PSUM->SBUF casts alternating between DVE and
    ACT (both engines ~3.2 us/pair -- the copy stream and the ~358 GB/s
    HBM write wire are the co-pacers of the kernel).
  - Output flushes: per m-tile one 512 KB chunk (cols 0:2048 = exactly
    4 KB/partition descriptors) + the 64 KB tail; even tiles on the SP
    ring, odd on the ACT ring so each engine issues one DMA per pair and
    waits only on its own side's copies.
"""

import numpy as np
import ml_dtypes

import concourse.bacc as bacc
import concourse.mybir as mybir
import concourse.tile as tile
from concourse.bass_interp import get_hw_module
from concourse.bass_utils import run_bass_kernel_spmd

B, H, W, C = 8, 48, 48, 64
HW = H * W  # 2304
P = 128
M_TILES = HW // P  # 18
M_PAIRS = M_TILES // 2  # 9
N_TILE = 512
FP32 = mybir.dt.float32
BF16 = mybir.dt.bfloat16
BF16_NP = ml_dtypes.bfloat16

N_SPLITS = []
_n0 = 0
while _n0 < HW:
    N_SPLITS.append((_n0, min(N_TILE, HW - _n0)))
    _n0 += N_TILE


def _corr_body(tc, out, a_hi, b_hi, b_lo):
    nc = tc.nc
    with (
        tc.tile_pool(name="ops", bufs=1) as op_pool,
        tc.tile_pool(name="ps", bufs=8, space="PSUM") as ps_pool,
        tc.tile_pool(name="outs", bufs=12) as out_pool,
    ):
        # lhsT operand: [128, 1152]; rows 0:64 = even m-tiles, 64:128 = odd
        ath = op_pool.tile([P, HW // 2], BF16)
        # rhs operands: [128, 2304]; rows 64:128 duplicate rows 0:64
        bth = op_pool.tile([P, HW], BF16)
        btl = op_pool.tile([P, HW], BF16)
        # Inputs ride the HWDGE rings (first byte ~0.6 us after the Tile
        # preamble vs ~9 us for the SWDGE/gpsimd path, and ~360 vs ~230
        # GB/s). Each dma_start costs ~0.6-0.7 us of issuing-engine time,
        # so split the issues across BOTH rings: sync takes the hh-term
        # operands (needed first), scalar takes the hl-term ones. Outputs
        # are issued later on the same rings (FIFO per ring, no conflict).
        for t, src, c0, c1 in [
            (ath, a_hi, 0, P),
            (bth, b_hi, 0, N_TILE),
            (ath, a_hi, P, HW // 2),
            (bth, b_hi, N_TILE, HW),
        ]:
            nc.sync.dma_start(out=t[:, c0:c1], in_=src[:, c0:c1])
        for t, src, c0, c1 in [
            (btl, b_lo, 0, N_TILE),
            (btl, b_lo, N_TILE, HW),
        ]:
            nc.scalar.dma_start(out=t[:, c0:c1], in_=src[:, c0:c1])

        for p in range(M_PAIRS):
            ot_e = out_pool.tile([P, HW], BF16, tag="ot")
            ot_o = out_pool.tile([P, HW], BF16, tag="ot")
            col = slice(p * P, (p + 1) * P)
            for ni, (n0, nsz) in enumerate(N_SPLITS):
                ps_e = ps_pool.tile([P, N_TILE], FP32, tag="ps")
                ps_o = ps_pool.tile([P, N_TILE], FP32, tag="ps")
                for k, (at, bt) in enumerate(((ath, bth), (ath, btl))):
                    st, sp = k == 0, k == 1
                    nc.tensor.matmul(
                        ps_e[:, :nsz],
                        at[0:64, col],
                        bt[0:64, n0 : n0 + nsz],
                        start=st,
                        stop=sp,
                    )
                    nc.tensor.matmul(
                        ps_o[:, :nsz],
                        at[64:128, col],
                        bt[64:128, n0 : n0 + nsz],
                        start=st,
                        stop=sp,
                    )
                # balance the narrow PSUM->SBUF copies across DVE and ACT
                # (narrow [128,512] copies keep the PSUM-recycle dependency
                # chain short; wide 2-bank copies measured slower overall)
                if ni % 2 == 0:
                    nc.vector.tensor_copy(ot_e[:, n0 : n0 + nsz], ps_e[:, :nsz])
                    nc.scalar.copy(ot_o[:, n0 : n0 + nsz], ps_o[:, :nsz])
                else:
                    nc.scalar.copy(ot_e[:, n0 : n0 + nsz], ps_e[:, :nsz])
                    nc.vector.tensor_copy(ot_o[:, n0 : n0 + nsz], ps_o[:, :nsz])
                # Flush 512 KB chunks (cols 0:2048 = exactly 4 KB/partition;
                # 4608 B patterns split 4096+512 and drop to ~205 GB/s) plus
                # the 64 KB tails. Even tiles drain on the SP ring, odd on
                # the ACT ring (a dma_start can only target the issuing
                # engine's own ring): each ring gets one DMA per pair
                # continuously, halving issue cost and blocking waits per
                # engine. The first pair flushes at ni=1 to start the
                # stream early.
                # Mid-stream pairs flush the full 2304-col row in ONE DMA
                # per tile (2 issues/pair instead of 4; the 4608 B/partition
                # descriptor split drops those DMAs to ~205 GB/s, but each
                # ring has duty slack against the ~3.9 us/pair production
                # cadence, and the issuing engines are the scarce resource).
                # First and last pairs stay chunked: p0 to start the stream
                # early, p8 so the final drain after the last copy is small.
                if p == 0:
                    sched = {1: 0, 3: 2 * N_TILE, 4: 4 * N_TILE}
                elif p == M_PAIRS - 1:
                    sched = {3: 0, 4: 4 * N_TILE}
                else:
                    sched = {4: 0}
                if ni in sched:
                    c0 = sched[ni]
                    c1 = n0 + nsz
                    m_e, m_o = 2 * p, 2 * p + 1
                    nc.sync.dma_start(
                        out=out[m_e * P : (m_e + 1) * P, c0:c1],
                        in_=ot_e[:, c0:c1],
                    )
                    nc.scalar.dma_start(
                        out=out[m_o * P : (m_o + 1) * P, c0:c1],
                        in_=ot_o[:, c0:c1],
                    )


_NC_CACHE = None


def _build():
    global _NC_CACHE
    if _NC_CACHE is None:
        nc = bacc.Bacc(
            "TRN2",
            target_bir_lowering=False,
            debug=False,
            enable_asserts=False,
        )
        a_hi = nc.dram_tensor("a_hi", [P, HW // 2], BF16, kind="ExternalInput").ap()
        b_hi = nc.dram_tensor("b_hi", [P, HW], BF16, kind="ExternalInput").ap()
        b_lo = nc.dram_tensor("b_lo", [P, HW], BF16, kind="ExternalInput").ap()
        out = nc.dram_tensor("out", [HW, HW], BF16, kind="ExternalOutput").ap()
        with tile.TileContext(nc) as tc:
            _corr_body(tc, out, a_hi, b_hi, b_lo)
        nc.compile()
        nc.m = get_hw_module(nc.m)
        _NC_CACHE = nc
    return _NC_CACHE


def _split_hi_lo(x):
    """x: [HW, C] fp32 -> (hi, lo) bf16 with x ~= hi + lo."""
    hi = x.astype(BF16_NP)
    lo = (x - hi.astype(np.float32)).astype(BF16_NP)
    return hi, lo


def _pack_lhs(xT):
    """[C, HW] -> [128, HW/2]: rows 0:64 even m-tiles, rows 64:128 odd."""
    t = xT.reshape(C, M_PAIRS, 2, P)  # [c, pair, eo, j]
    return np.ascontiguousarray(t.transpose(2, 0, 1, 3).reshape(2 * C, M_PAIRS * P))


def _pack_rhs(xT):
    """[C, HW] -> [128, HW]: duplicate into both partition halves."""
    return np.ascontiguousarray(np.concatenate([xT, xT], axis=0))


def _prep_inputs(feature_A, feature_B):
    in_maps = []
    for i in range(B):
        A2 = np.ascontiguousarray(feature_A[i].reshape(HW, C), dtype=np.float32)
        B2 = np.ascontiguousarray(feature_B[i].reshape(HW, C), dtype=np.float32)
        ah, _ = _split_hi_lo(A2)
        bh, bl = _split_hi_lo(B2)
        in_maps.append(
            {
                "a_hi": _pack_lhs(np.ascontiguousarray(ah.T)),
                "b_hi": _pack_rhs(np.ascontiguousarray(bh.T)),
                "b_lo": _pack_rhs(np.ascontiguousarray(bl.T)),
            }
        )
    return in_maps


def _run(feature_A, feature_B, trace=False, **kwargs):
    feature_A = np.asarray(feature_A, dtype=np.float32)
    feature_B = np.asarray(feature_B, dtype=np.float32)
    assert feature_A.shape == (B, H, W, C), feature_A.shape
    assert feature_B.shape == (B, H, W, C), feature_B.shape

    nc = _build()
    in_maps = _prep_inputs(feature_A, feature_B)
    res = run_bass_kernel_spmd(nc, in_maps, list(range(B)), trace=trace, **kwargs)
    out = np.stack(
        [np.asarray(res.results[i]["out"]).astype(np.float32) for i in range(B)],
        axis=0,
    )
    return out.reshape(B, H, W, H, W), res


def kernel(feature_A, feature_B):
    out, _ = _run(feature_A, feature_B)
    return out



# revision 2
# speedup vs baseline: 1.1422x; 1.1422x over previous
"""Correlation network kernel for Trainium2.

corr[b,i,j,k,l] = sum_c A[b,i,j,c] * B[b,k,l,c]

Per batch b this is  A_b (2304x64) @ B_b^T (64x2304) -> 2304x2304.
Sharding: data-parallel over batch B=8 across the 8 NeuronCores; each core
computes one full 2304x2304 correlation matrix. Output is emitted as BF16
(10.6 MB/core; host upcasts), so the kernel floor is the ~358 GB/s HBM
write wire (~29.6 us) plus the ~7.3 us Tile preamble.

v2 design (vs the 52-59 us baseline):
  - K=128 stacking: lhsT rows 0:64 = A_hi^T, rows 64:128 = A_lo^T
    (A = A_hi + A_lo split in bf16); rhs rows = B_hi^T duplicated.
    One full-array matmul then computes (A_hi+A_lo)@B_hi = A@B_hi in a
    single pass -- 2x fewer PE column-streams than the even/odd K=64
    packing, and 1 matmul per (m-tile, bank) instead of 4.
    Error = A@(B - B_hi) ~ 2.3e-3 fro, same class as before.
  - Wide PSUM->SBUF casts: [128,1024] (2-bank) copies on DVE/ACT
    amortize the per-instruction overhead; 3 copies per m-tile
    (1024/1024/256) alternating engines, ~1.3-1.6 us/engine/m-tile,
    under the 1.65 us/m-tile write pace.
  - Output DRAM layout [1152, 4608]: pair p of m-tiles (2p, 2p+1) lives
    in rows p*128..(p+1)*128 as [even cols 0:2304 | odd cols 2304:4608],
    i.e. a verbatim dump of the SBUF staging tile. Each pair flushes as
    ONE 1.18 MB DMA with fully linear 9216 B/partition descriptors
    (no 2D pattern) on the sync HWDGE ring. Host reassembles.
  - PSUM: 3x [128,1024] + 2x [128,512] pools = exactly 8 banks.
"""

import numpy as np
import ml_dtypes

import concourse.bacc as bacc
import concourse.mybir as mybir
import concourse.tile as tile
from concourse.bass_interp import get_hw_module
from concourse.bass_utils import run_bass_kernel_spmd

B, H, W, C = 8, 48, 48, 64
HW = H * W  # 2304
P = 128
M_TILES = HW // P  # 18
M_PAIRS = M_TILES // 2  # 9
FP32 = mybir.dt.float32
BF16 = mybir.dt.bfloat16
BF16_NP = ml_dtypes.bfloat16


def _corr_body(tc, out, lhs_h, rhs_h):
    nc = tc.nc
    with (
        tc.tile_pool(name="ops", bufs=1) as op_pool,
        tc.tile_pool(name="psw", bufs=3, space="PSUM") as ps_wide,
        tc.tile_pool(name="pst", bufs=2, space="PSUM") as ps_tail,
        tc.tile_pool(name="outs", bufs=3) as out_pool,
    ):
        lt = op_pool.tile([P, HW], BF16)
        rt = op_pool.tile([P, HW], BF16)
        # Input loads: split across both HWDGE rings; first chunks cover
        # the first m-tiles so the PE starts ~1 us after the preamble.
        nc.sync.dma_start(out=rt[:, 0:1152], in_=rhs_h[:, 0:1152])
        nc.sync.dma_start(out=lt[:, 0:256], in_=lhs_h[:, 0:256])
        nc.scalar.dma_start(out=rt[:, 1152:HW], in_=rhs_h[:, 1152:HW])
        nc.scalar.dma_start(out=lt[:, 256:HW], in_=lhs_h[:, 256:HW])

        for m in range(M_TILES):
            p, eo = divmod(m, 2)
            if eo == 0:
                ot = out_pool.tile([P, 2 * HW], BF16, tag="ot")
            base = eo * HW
            lcol = lt[:, m * P : (m + 1) * P]
            t0 = ps_wide.tile([P, 1024], FP32, tag="ps")
            t1 = ps_wide.tile([P, 1024], FP32, tag="ps")
            t2 = ps_tail.tile([P, 512], FP32, tag="pt")
            nc.tensor.matmul(t0[:, 0:512], lcol, rt[:, 0:512], start=True, stop=True)
            nc.tensor.matmul(t0[:, 512:1024], lcol, rt[:, 512:1024], start=True, stop=True)
            nc.tensor.matmul(t1[:, 0:512], lcol, rt[:, 1024:1536], start=True, stop=True)
            nc.tensor.matmul(t1[:, 512:1024], lcol, rt[:, 1536:2048], start=True, stop=True)
            nc.tensor.matmul(t2[:, 0:256], lcol, rt[:, 2048:2304], start=True, stop=True)
            # 3 copies per m-tile; alternate which engine takes the pair
            # {1024} vs {1024, 256} so DVE/ACT stay balanced.
            if m % 2 == 0:
                nc.vector.tensor_copy(ot[:, base : base + 1024], t0[:, :])
                nc.scalar.copy(ot[:, base + 1024 : base + 2048], t1[:, :])
                nc.scalar.copy(ot[:, base + 2048 : base + 2304], t2[:, 0:256])
            else:
                nc.scalar.copy(ot[:, base : base + 1024], t0[:, :])
                nc.vector.tensor_copy(ot[:, base + 1024 : base + 2048], t1[:, :])
                nc.vector.tensor_copy(ot[:, base + 2048 : base + 2304], t2[:, 0:256])
            if p == 0:
                # start the write stream as soon as the first m-tile lands
                nc.sync.dma_start(
                    out=out[0:P, base : base + HW], in_=ot[:, base : base + HW]
                )
            elif eo == 1:
                nc.sync.dma_start(
                    out=out[p * P : (p + 1) * P, :], in_=ot[:, :]
                )


_NC_CACHE = None


def _build():
    global _NC_CACHE
    if _NC_CACHE is None:
        nc = bacc.Bacc(
            "TRN2",
            target_bir_lowering=False,
            debug=False,
            enable_asserts=False,
        )
        lhs_h = nc.dram_tensor("lhs_h", [P, HW], BF16, kind="ExternalInput").ap()
        rhs_h = nc.dram_tensor("rhs_h", [P, HW], BF16, kind="ExternalInput").ap()
        out = nc.dram_tensor("out", [M_PAIRS * P, 2 * HW], BF16, kind="ExternalOutput").ap()
        with tile.TileContext(nc) as tc:
            _corr_body(tc, out, lhs_h, rhs_h)
        nc.compile()
        nc.m = get_hw_module(nc.m)
        _NC_CACHE = nc
    return _NC_CACHE


def _prep_inputs(feature_A, feature_B):
    in_maps = []
    for i in range(B):
        A2 = np.ascontiguousarray(feature_A[i].reshape(HW, C), dtype=np.float32)
        B2 = np.ascontiguousarray(feature_B[i].reshape(HW, C), dtype=np.float32)
        ah = A2.astype(BF16_NP)
        al = (A2 - ah.astype(np.float32)).astype(BF16_NP)
        bh = B2.astype(BF16_NP)
        # lhsT [128, 2304]: rows 0:64 = A_hi^T, rows 64:128 = A_lo^T
        lhs = np.concatenate([ah.T, al.T], axis=0)
        # rhs [128, 2304]: B_hi^T duplicated into both partition halves
        rhs = np.concatenate([bh.T, bh.T], axis=0)
        in_maps.append(
            {
                "lhs_h": np.ascontiguousarray(lhs),
                "rhs_h": np.ascontiguousarray(rhs),
            }
        )
    return in_maps


def _unpack_out(o):
    """[1152, 4608] pair-packed -> [2304, 2304] fp32."""
    o = np.asarray(o).reshape(M_PAIRS, P, 2, HW)
    return o.transpose(0, 2, 1, 3).reshape(HW, HW).astype(np.float32)


def _run(feature_A, feature_B, trace=False, **kwargs):
    feature_A = np.asarray(feature_A, dtype=np.float32)
    feature_B = np.asarray(feature_B, dtype=np.float32)
    assert feature_A.shape == (B, H, W, C), feature_A.shape
    assert feature_B.shape == (B, H, W, C), feature_B.shape

    nc = _build()
    in_maps = _prep_inputs(feature_A, feature_B)
    res = run_bass_kernel_spmd(nc, in_maps, list(range(B)), trace=trace, **kwargs)
    out = np.stack([_unpack_out(res.results[i]["out"]) for i in range(B)], axis=0)
    return out.reshape(B, H, W, H, W), res


def kernel(feature_A, feature_B):
    out, _ = _run(feature_A, feature_B)
    return out


# revision 3
# speedup vs baseline: 1.1485x; 1.0055x over previous
"""Correlation network kernel for Trainium2.

corr[b,i,j,k,l] = sum_c A[b,i,j,c] * B[b,k,l,c]

Per batch b this is  A_b (2304x64) @ B_b^T (64x2304) -> 2304x2304.
Sharding: data-parallel over batch B=8 across the 8 NeuronCores; each core
computes one full 2304x2304 correlation matrix. Output is emitted as BF16
(10.6 MB/core; host upcasts), so the kernel floor is the ~358 GB/s HBM
write wire (~30 us) plus the ~7.3 us Tile preamble.

v3 design:
  - fp8 e4m3 DoubleRow matmuls: A = A_hi + A_lo and B = B_hi + B_lo
    (fp8 hi/lo splits, residual ~7e-4 relative). K=256 packed as 2
    k-values per partition: partition p holds channel c = p%64 of
    A_hi (p<64) / A_lo (p>=64); k-tile 0 pairs with B_hi, k-tile 1
    with B_lo. One DoubleRow matmul per (m-tile, n-bank) computes the
    full (A_hi+A_lo)@(B_hi+B_lo) product at 0.5 cycles/output column
    -- 2x the bf16 column rate, PE ~15 us, well off the critical path.
    fro err ~1e-3, better than the bf16 hi*hi+hi*lo baseline (2.3e-3).
  - Wide PSUM->SBUF casts: [128,1024] (2-bank) copies alternating
    DVE/ACT (~1.4 us/engine/m-tile), under the 1.7 us/m-tile write pace.
    PSUM = 3x 2-bank + 2x 1-bank pools = exactly 8 banks.
  - Output DRAM layout [1152, 4608]: pair p of m-tiles (2p, 2p+1) lives
    in rows p*128..(p+1)*128 as [even cols 0:2304 | odd cols 2304:4608],
    a verbatim dump of the SBUF staging tile; host reassembles. Each
    m-tile flushes as one 0.59 MB DMA (4608 B/partition) on the sync
    HWDGE ring; the last m-tile flushes in two chunks to cut the drain.
"""

import numpy as np
import ml_dtypes

import concourse.bacc as bacc
import concourse.mybir as mybir
import concourse.tile as tile
from concourse.bass_interp import get_hw_module
from concourse.bass_utils import run_bass_kernel_spmd

B, H, W, C = 8, 48, 48, 64
HW = H * W  # 2304
P = 128
M_TILES = HW // P  # 18
M_PAIRS = M_TILES // 2  # 9
FP32 = mybir.dt.float32
BF16 = mybir.dt.bfloat16
FP8 = mybir.dt.float8e4
BF16_NP = ml_dtypes.bfloat16
FP8_NP = ml_dtypes.float8_e4m3
DR = mybir.MatmulPerfMode.DoubleRow


def _corr_body(tc, out, lhs_h, rhs_h):
    nc = tc.nc
    with (
        tc.tile_pool(name="ops", bufs=1) as op_pool,
        tc.tile_pool(name="psw", bufs=3, space="PSUM") as ps_wide,
        tc.tile_pool(name="pst", bufs=2, space="PSUM") as ps_tail,
        tc.tile_pool(name="outs", bufs=3) as out_pool,
    ):
        lt = op_pool.tile([P, 2, HW], FP8)
        rt = op_pool.tile([P, 2, HW], FP8)
        # Input loads: first chunks cover the first m-tiles so the PE
        # starts right after the preamble; rest streams on both rings.
        nc.sync.dma_start(out=lt[:, :, 0:256], in_=lhs_h[:, :, 0:256])
        nc.sync.dma_start(out=rt[:, :, 0:1152], in_=rhs_h[:, :, 0:1152])
        nc.scalar.dma_start(out=rt[:, :, 1152:HW], in_=rhs_h[:, :, 1152:HW])
        nc.scalar.dma_start(out=lt[:, :, 256:HW], in_=lhs_h[:, :, 256:HW])

        for m in range(M_TILES):
            p, eo = divmod(m, 2)
            if eo == 0:
                ot = out_pool.tile([P, 2 * HW], BF16, tag="ot")
            base = eo * HW
            lcol = lt[:, :, m * P : (m + 1) * P]
            t0 = ps_wide.tile([P, 1024], FP32, tag="ps")
            t1 = ps_wide.tile([P, 1024], FP32, tag="ps")
            t2 = ps_tail.tile([P, 512], FP32, tag="pt")
            for ps, o0, o1 in (
                (t0[:, 0:512], 0, 512),
                (t0[:, 512:1024], 512, 1024),
                (t1[:, 0:512], 1024, 1536),
                (t1[:, 512:1024], 1536, 2048),
                (t2[:, 0:256], 2048, 2304),
            ):
                nc.tensor.matmul(
                    ps, lcol, rt[:, :, o0:o1], start=True, stop=True, perf_mode=DR
                )
            # 3 copies per m-tile; alternate which engine takes the pair
            # {1024} vs {1024, 256} so DVE/ACT stay balanced.
            if m % 2 == 0:
                nc.vector.tensor_copy(ot[:, base : base + 1024], t0[:, :])
                nc.scalar.copy(ot[:, base + 1024 : base + 2048], t1[:, :])
                nc.scalar.copy(ot[:, base + 2048 : base + 2304], t2[:, 0:256])
            else:
                nc.scalar.copy(ot[:, base : base + 1024], t0[:, :])
                nc.vector.tensor_copy(ot[:, base + 1024 : base + 2048], t1[:, :])
                nc.vector.tensor_copy(ot[:, base + 2048 : base + 2304], t2[:, 0:256])
            # one 0.59 MB linear flush per m-tile; split the last one so
            # the final drain after the last copy is small
            if m == M_TILES - 1:
                nc.sync.dma_start(
                    out=out[p * P : (p + 1) * P, base : base + 1024],
                    in_=ot[:, base : base + 1024],
                )
                nc.sync.dma_start(
                    out=out[p * P : (p + 1) * P, base + 1024 : base + HW],
                    in_=ot[:, base + 1024 : base + HW],
                )
            else:
                nc.sync.dma_start(
                    out=out[p * P : (p + 1) * P, base : base + HW],
                    in_=ot[:, base : base + HW],
                )


_NC_CACHE = None


def _build():
    global _NC_CACHE
    if _NC_CACHE is None:
        nc = bacc.Bacc(
            "TRN2",
            target_bir_lowering=False,
            debug=False,
            enable_asserts=False,
        )
        lhs_h = nc.dram_tensor("lhs_h", [P, 2, HW], FP8, kind="ExternalInput").ap()
        rhs_h = nc.dram_tensor("rhs_h", [P, 2, HW], FP8, kind="ExternalInput").ap()
        out = nc.dram_tensor("out", [M_PAIRS * P, 2 * HW], BF16, kind="ExternalOutput").ap()
        with tile.TileContext(nc) as tc:
            _corr_body(tc, out, lhs_h, rhs_h)
        nc.compile()
        nc.m = get_hw_module(nc.m)
        _NC_CACHE = nc
    return _NC_CACHE


def _prep_inputs(feature_A, feature_B):
    in_maps = []
    for i in range(B):
        A2 = np.ascontiguousarray(feature_A[i].reshape(HW, C), dtype=np.float32)
        B2 = np.ascontiguousarray(feature_B[i].reshape(HW, C), dtype=np.float32)
        ah = A2.astype(FP8_NP)
        al = (A2 - ah.astype(np.float32)).astype(FP8_NP)
        bh = B2.astype(FP8_NP)
        bl = (B2 - bh.astype(np.float32)).astype(FP8_NP)
        # lhs [128, 2, 2304]: partition p<64 = A_hi ch p, p>=64 = A_lo
        # ch p-64; identical across the two k-tiles.
        apart = np.concatenate([ah.T, al.T], axis=0)  # [128, 2304]
        lhs = np.stack([apart, apart], axis=1)  # [128, 2, 2304]
        # rhs [128, 2, 2304]: k-tile 0 = B_hi ch p%64, k-tile 1 = B_lo.
        r0 = np.concatenate([bh.T, bh.T], axis=0)
        r1 = np.concatenate([bl.T, bl.T], axis=0)
        rhs = np.stack([r0, r1], axis=1)
        in_maps.append(
            {
                "lhs_h": np.ascontiguousarray(lhs),
                "rhs_h": np.ascontiguousarray(rhs),
            }
        )
    return in_maps


def _unpack_out(o):
    """[1152, 4608] pair-packed -> [2304, 2304] fp32."""
    o = np.asarray(o).reshape(M_PAIRS, P, 2, HW)
    return o.transpose(0, 2, 1, 3).reshape(HW, HW).astype(np.float32)


def _run(feature_A, feature_B, trace=False, **kwargs):
    feature_A = np.asarray(feature_A, dtype=np.float32)
    feature_B = np.asarray(feature_B, dtype=np.float32)
    assert feature_A.shape == (B, H, W, C), feature_A.shape
    assert feature_B.shape == (B, H, W, C), feature_B.shape

    nc = _build()
    in_maps = _prep_inputs(feature_A, feature_B)
    res = run_bass_kernel_spmd(nc, in_maps, list(range(B)), trace=trace, **kwargs)
    out = np.stack([_unpack_out(res.results[i]["out"]) for i in range(B)], axis=0)
    return out.reshape(B, H, W, H, W), res


def kernel(feature_A, feature_B):
    out, _ = _run(feature_A, feature_B)
    return out
